# revision 1
# baseline (speedup 1.0000x reference)
"""Trainium2 Bass kernel v2 for nn_Encoder (DA-RNN input-attention encoder).

Math (identical to baseline's rewrite):
  alpha = softmax_n(score_x),  score_x[b,n] = sum_t X[b,t,n] w_x[t]
  X_tilde = alpha[:,None,:] * X
  xb = BN(X_tilde) (batch stats over full B -> one AllReduce)
  X_encoded = 2-layer LSTM over xb

v2 structural changes vs baseline:
  - X is cast+permuted once to a (T, BL, N) fp16 DRAM scratch by pure DMA.
  - The (n, t, b) fp16 cache is built by DMA xbar *transposed loads* from the
    scratch: zero PE transposes, zero PSUM->SBUF copies on ACT/DVE.
  - score_x is accumulated from the cache (DVE/Pool split), softmax via 8 PE
    transposes of 128x128 blocks.
  - BN scale is bulk-multiplied into the cache; BN shift is folded into a
    per-(gate,t) bias table via 4 tiny matmuls (bias rides the ACT bias port).
  - LSTM keeps all 10 transcendentals/step on ACT (hard floor ~4.6us/step),
    cheap tails on DVE in fp16, XT pass on GPSIMD, everything else under.
"""

import sys

sys.path.insert(0, "/opt/trn_rl_repo")

import numpy as np

import concourse.bass as bass
import concourse.bacc as bacc
import concourse.tile as tile
import concourse.mybir as mybir
from concourse import masks
from concourse.alu_op_type import AluOpType
from concourse.bass_utils import run_bass_kernel_spmd

FP32 = mybir.dt.float32
CDT = mybir.dt.float16
AF = mybir.ActivationFunctionType

B, T, N, H = 4096, 128, 128, 128
EPS = 1e-5
NCORES = 8
BL = B // NCORES
NB = BL // 128
TC_E = 2        # timesteps per X_encoded staging flush
TC_X = 4        # timesteps per XT-pass chunk


def build_nc(ncores=NCORES, bl=BL, t_len=T, collective=True):
    nb = bl // 128
    nc = bacc.Bacc("TRN2", target_bir_lowering=False, debug=False,
                   num_devices=ncores)

    X_d = nc.dram_tensor("x_in", (bl, t_len, N), FP32, kind="ExternalInput")
    wxb_d = nc.dram_tensor("wxb", (128, t_len), FP32, kind="ExternalInput")
    gam_d = nc.dram_tensor("gamma_c", (N, 1), FP32, kind="ExternalInput")
    bet_d = nc.dram_tensor("beta_c", (N, 1), FP32, kind="ExternalInput")
    w0i_d = nc.dram_tensor("w0i_t", (N, 4 * H), FP32, kind="ExternalInput")
    w0h_d = nc.dram_tensor("w0h_t", (H, 4 * H), FP32, kind="ExternalInput")
    w1i_d = nc.dram_tensor("w1i_t", (H, 4 * H), FP32, kind="ExternalInput")
    w1h_d = nc.dram_tensor("w1h_t", (H, 4 * H), FP32, kind="ExternalInput")
    b0_d = nc.dram_tensor("b0_c", (128, 4), FP32, kind="ExternalInput")
    b1_d = nc.dram_tensor("b1_c", (128, 4), FP32, kind="ExternalInput")

    XT_d = nc.dram_tensor("xt_out", (bl, t_len, N), FP32, kind="ExternalOutput")
    XE_d = nc.dram_tensor("xe_out", (bl, t_len, H), FP32, kind="ExternalOutput")

    Xap = X_d.ap()
    XTap = XT_d.ap()
    XEap = XE_d.ap().rearrange("(q p) t h -> p q t h", p=128)

    with tile.TileContext(nc) as tc:
        with (
            tc.tile_pool(name="consts", bufs=1) as consts,
            tc.tile_pool(name="cachep", bufs=1) as cachep,
            tc.tile_pool(name="smallp", bufs=1) as smallp,
            tc.tile_pool(name="xtin", bufs=3) as xtin,
            tc.tile_pool(name="xtout", bufs=3) as xtout,
            tc.tile_pool(name="stageE", bufs=3) as stageE,
            tc.tile_pool(name="gates", bufs=2) as gatesp,
            tc.tile_pool(name="tiny", bufs=2) as tiny,
            tc.tile_pool(name="psum", bufs=7, space="PSUM") as psump,
            tc.tile_pool(name="psumE", bufs=1, space="PSUM") as psumE,
            tc.tile_pool(name="dram", bufs=1, space="DRAM") as dramp,
        ):
            # ---------------- constants ----------------
            ident_f = consts.tile([128, 128], FP32)
            masks.make_identity(nc, ident_f[:])
            ident_b = consts.tile([128, 128], CDT)
            masks.make_identity(nc, ident_b[:])

            wxb = consts.tile([128, t_len], FP32)
            nc.sync.dma_start(wxb[:], wxb_d.ap())
            gammaC = consts.tile([N, 1], FP32)
            nc.sync.dma_start(gammaC[:], gam_d.ap())
            betaC = consts.tile([N, 1], FP32)
            nc.sync.dma_start(betaC[:], bet_d.ap())
            b0c = consts.tile([128, 4], FP32)
            nc.sync.dma_start(b0c[:], b0_d.ap())
            b1c = consts.tile([128, 4], FP32)
            nc.sync.dma_start(b1c[:], b1_d.ap())
            W = {}
            for nm, d in (("w0i", w0i_d), ("w0h", w0h_d),
                          ("w1i", w1i_d), ("w1h", w1h_d)):
                wt = consts.tile([128, 4 * H], CDT, tag=f"W_{nm}", name=f"W_{nm}")
                nc.gpsimd.dma_start(wt[:], d.ap())
                W[nm] = wt

            # ---------------- persistent tiles ----------------
            cache = cachep.tile([128, t_len, bl], CDT)   # X^T then xtilde^T then xb^T
            P1C = 16
            xscr_c = [dramp.tile([bl, P1C, N], CDT, name=f"xscr{c}")
                      for c in range(t_len // P1C)]

            scoreT = [smallp.tile([128, bl], FP32, tag=f"scT{i}",
                                  name=f"scT{i}") for i in range(2)]
            alphaT = smallp.tile([128, bl], CDT, tag="alphaT", name="alphaT")
            alphaB = smallp.tile([128, nb, 128], CDT, tag="alphaB", name="alphaB")
            bn6 = smallp.tile([128, t_len, 6], FP32)
            scaleT = smallp.tile([128, t_len], FP32, tag="scaleT")
            shiftT16 = smallp.tile([128, t_len], CDT, tag="shiftT16")
            scaleT16 = smallp.tile([128, t_len], CDT, tag="scaleT16")
            biasT = smallp.tile([128, 4, t_len], FP32, tag="biasT")

            # ============ P1: cast X -> xscr (pure DMA, contiguous) ========
            for c0 in range(t_len // P1C):
                tsl = slice(c0 * P1C, (c0 + 1) * P1C)
                nc.gpsimd.dma_start(xscr_c[c0][:], Xap[:, tsl, :])

            # ====== P2: transposed loads + P3: score accumulation ==========
            nc.vector.memset(scoreT[0][:], 0.0)
            for t in range(t_len):
                nc.sync.dma_start(cache[:, t, :], xscr_c[t // P1C][:, t % P1C, :],
                                  transpose=True)
            for t in range(t_len):
                nc.vector.scalar_tensor_tensor(
                    scoreT[0][:], cache[:, t, :], wxb[:, t:t + 1],
                    scoreT[0][:], AluOpType.mult, AluOpType.add)

            # ============ P4: softmax over n -> alphaT / alphaB ============
            for q in range(nb):
                ps = psump.tile([128, 128], FP32, tag="ps", name="ps_sm")
                nc.tensor.transpose(ps[:], scoreT[0][:, q * 128:(q + 1) * 128],
                                    ident_f[:])
                sq = tiny.tile([128, 128], FP32, tag="sq", name="sq")
                nmax = tiny.tile([128, 1], FP32, tag="nmax")
                nc.vector.reduce_max(nmax[:], ps[:],
                                     axis=mybir.AxisListType.X, negate=True)
                sume = tiny.tile([128, 1], FP32, tag="sume")
                nc.scalar.activation(sq[:], ps[:], AF.Exp,
                                     bias=nmax[:], scale=1.0, accum_out=sume[:])
                rec = tiny.tile([128, 1], FP32, tag="rec")
                nc.vector.reciprocal(rec[:], sume[:])
                nc.vector.tensor_scalar_mul(sq[:], sq[:], rec[:])
                nc.vector.tensor_copy(alphaB[:, q, :], sq[:])
                psb = psump.tile([128, 128], CDT, tag="ps", name="ps_smb")
                nc.tensor.transpose(psb[:], alphaB[:, q, :], ident_b[:])
                nc.vector.tensor_copy(alphaT[:, q * 128:(q + 1) * 128], psb[:])

            # ======= P5..P8 per t-half: alpha-mult, stats, AR, scale =======
            def half_prologue(hf, t_lo, t_hi, bulk_eng=None):
                """Build the per-quarter normalization pipeline as a list of
                thunks so it can be interleaved into the LSTM loop."""
                if bulk_eng is None:
                    bulk_eng = nc.vector
                thunks = []
                ACH = 8
                for c0 in range(t_lo, t_hi, ACH):
                    def alpha_mult(c0=c0):
                        for t in range(c0, c0 + ACH):
                            bulk_eng.tensor_tensor(
                                cache[:, t, :], cache[:, t, :], alphaT[:],
                                AluOpType.mult)
                        for t in range(c0, c0 + ACH):
                            nc.vector.bn_stats(bn6[:, t, :], cache[:, t, :])
                    thunks.append(alpha_mult)

                tn = t_hi - t_lo
                m_e = bn6[:, t_lo:t_hi, 1]
                m_o = bn6[:, t_lo:t_hi, 4]
                cv_e = bn6[:, t_lo:t_hi, 2]
                cv_o = bn6[:, t_lo:t_hi, 5]
                Spack = smallp.tile([128, 2, tn], FP32, tag=f"Spack{hf}",
                                    name=f"Spack{hf}")

                def pack_and_allreduce():
                  tsum = smallp.tile([128, tn], FP32, tag="tsum", name="tsum",
                                     bufs=2)
                  nc.vector.tensor_tensor(tsum[:], m_e, m_o, AluOpType.add)
                  half_n = float(bl // 2)
                  nc.vector.tensor_scalar_mul(Spack[:, 0, :], tsum[:], half_n)
                  sq_e = smallp.tile([128, tn], FP32, tag="sq_e", name="sq_e",
                                     bufs=2)
                  nc.vector.tensor_tensor(sq_e[:], m_e, m_e, AluOpType.mult)
                  sq_o = smallp.tile([128, tn], FP32, tag="sq_o", name="sq_o",
                                     bufs=2)
                  nc.vector.tensor_tensor(sq_o[:], m_o, m_o, AluOpType.mult)
                  nc.vector.tensor_tensor(sq_e[:], sq_e[:], sq_o[:],
                                          AluOpType.add)
                  cvs = smallp.tile([128, tn], FP32, tag="cvs", name="cvs",
                                    bufs=2)
                  nc.vector.tensor_tensor(cvs[:], cv_e, cv_o, AluOpType.add)
                  nc.vector.scalar_tensor_tensor(
                      Spack[:, 1, :], sq_e[:], half_n, cvs[:],
                      AluOpType.mult, AluOpType.add)

                  cc_in = dramp.tile([128, 2, tn], FP32, name=f"cc_in{hf}")
                  cc_out = dramp.tile([128, 2, tn], FP32, name=f"cc_out{hf}")
                  nc.gpsimd.dma_start(cc_in[:], Spack[:])
                  if collective:
                      nc.gpsimd.collective_compute(
                          "AllReduce", AluOpType.add,
                          replica_groups=[list(range(ncores))],
                          ins=[cc_in[:].opt()], outs=[cc_out[:].opt()])
                  else:
                      nc.gpsimd.dma_start(cc_out[:], cc_in[:])
                  nc.gpsimd.dma_start(Spack[:], cc_out[:])
                thunks.append(pack_and_allreduce)

                def stats_math():
                  inv_b = 1.0 / float(bl * ncores)
                  mean = smallp.tile([128, tn], FP32, tag="mean", name="mean",
                                     bufs=2)
                  nc.vector.tensor_scalar_mul(mean[:], Spack[:, 0, :], inv_b)
                  m2 = smallp.tile([128, tn], FP32, tag="m2", name="m2", bufs=2)
                  nc.vector.tensor_tensor(m2[:], mean[:], mean[:],
                                          AluOpType.mult)
                  ve = smallp.tile([128, tn], FP32, tag="ve", name="ve", bufs=2)
                  nc.vector.scalar_tensor_tensor(
                      ve[:], Spack[:, 1, :], inv_b, m2[:],
                      AluOpType.mult, AluOpType.subtract)
                  nc.vector.tensor_scalar_add(ve[:], ve[:], EPS)
                  stdt = smallp.tile([128, tn], FP32, tag="stdt", name="stdt",
                                     bufs=2)
                  nc.scalar.activation(stdt[:], ve[:], AF.Sqrt)
                  r = smallp.tile([128, tn], FP32, tag="r", name="r", bufs=2)
                  nc.vector.reciprocal(r[:], stdt[:])
                  rr = smallp.tile([128, tn], FP32, tag="rr", name="rr", bufs=2)
                  nc.vector.tensor_tensor(rr[:], r[:], r[:], AluOpType.mult)
                  nc.vector.tensor_tensor(rr[:], rr[:], ve[:], AluOpType.mult)
                  nc.vector.tensor_scalar(rr[:], rr[:], -0.5, 1.5,
                                          AluOpType.mult, AluOpType.add)
                  nc.vector.tensor_tensor(r[:], r[:], rr[:], AluOpType.mult)
                  nc.vector.tensor_scalar_mul(scaleT[:, t_lo:t_hi], r[:],
                                              gammaC[:])
                  nc.vector.tensor_copy(scaleT16[:, t_lo:t_hi],
                                        scaleT[:, t_lo:t_hi])
                  # shift = beta - mean*scale (fp16 for the bias matmul)
                  ms = smallp.tile([128, tn], FP32, tag="ms", name="ms", bufs=2)
                  nc.vector.tensor_tensor(ms[:], mean[:], scaleT[:, t_lo:t_hi],
                                          AluOpType.mult)
                  msn = smallp.tile([128, tn], FP32, tag="msn", name="msn",
                                    bufs=2)
                  nc.vector.tensor_scalar(msn[:], ms[:], -1.0, betaC[:],
                                          AluOpType.mult, AluOpType.add)
                  nc.vector.tensor_copy(shiftT16[:, t_lo:t_hi], msn[:])
                  # gate bias table: biasT[:, g, t] = W_ih0[:,g].T @ shift + b0
                  for g in range(4):
                      psb = psump.tile([128, tn], FP32, tag="ps",
                                       name="ps_bias")
                      nc.tensor.matmul(psb[:],
                                       W["w0i"][:, g * 128:(g + 1) * 128],
                                       shiftT16[:, t_lo:t_hi], start=True,
                                       stop=True, skip_group_check=True)
                      nc.vector.tensor_scalar_add(biasT[:, g, t_lo:t_hi],
                                                  psb[:], b0c[:, g:g + 1])
                thunks.append(stats_math)

                # bulk scale multiply into cache (fp16 2x)
                for c0 in range(t_lo, t_hi, ACH):
                    def scale_mult(c0=c0):
                        for t in range(c0, c0 + ACH):
                            nc.vector.tensor_scalar_mul(
                                cache[:, t, :], cache[:, t, :],
                                scaleT[:, t:t + 1])
                    thunks.append(scale_mult)
                return thunks

            # ================= XT pass (Pool + DMA) ========================
            def xt_chunk(c0):
                # fp32 staging so both DMAs are cast-free HWDGE (SP queue)
                t0 = c0 * TC_X
                tsl = slice(c0 * TC_X, (c0 + 1) * TC_X)
                scr = xscr_c[t0 // P1C]
                ssl = slice(t0 % P1C, t0 % P1C + TC_X)
                for q in range(nb):
                    xin = xtin.tile([128, TC_X, 128], CDT, tag="xin",
                                    name="xin")
                    nc.sync.dma_start(
                        xin[:], scr[q * 128:(q + 1) * 128, ssl, :])
                    al_b = alphaB[:, q, :].rearrange(
                        "p (o n) -> p o n", o=1).broadcast_to((128, TC_X, 128))
                    xo = xtout.tile([128, TC_X, 128], FP32, tag="xo",
                                    name="xo")
                    nc.gpsimd.tensor_tensor(xo[:], xin[:], al_b,
                                            AluOpType.mult)
                    nc.sync.dma_start(
                        XTap[q * 128:(q + 1) * 128, tsl, :], xo[:])

            # ===================== phase B: LSTM ===========================
            h0 = [smallp.tile([128, bl], CDT, tag=f"h0_{i}", name=f"h0_{i}")
                  for i in range(2)]
            c0s = [smallp.tile([128, bl], CDT, tag=f"c0_{i}", name=f"c0_{i}")
                   for i in range(2)]
            h1 = [smallp.tile([128, bl], CDT, tag=f"h1_{i}", name=f"h1_{i}")
                  for i in range(2)]
            c1s = [smallp.tile([128, bl], CDT, tag=f"c1_{i}", name=f"c1_{i}")
                   for i in range(2)]
            for tl in (h0[0], c0s[0], h1[0], c1s[0]):
                nc.vector.memset(tl[:], 0.0)

            stE_ref = [stageE.tile([128, nb, TC_E, 128], FP32, tag="stE",
                                   name="stE")]

            GATE_ORDER = (2, 0, 1, 3)  # g, i, f, o: close the c-path early

            def g0_ih(t):
                ps = {}
                for g in GATE_ORDER:
                    p = psump.tile([128, bl], FP32, tag="ps", name="ps0")
                    nc.tensor.matmul(p[:], W["w0i"][:, g * 128:(g + 1) * 128],
                                     cache[:, t, :], start=True, stop=False,
                                     skip_group_check=True)
                    ps[g] = p
                return ps

            def cell_tail(s, cprev, cnew, hnew, li):
                qq = gatesp.tile([128, bl], CDT, tag=f"qq{li}",
                                 name=f"qq{li}", bufs=1)
                nc.vector.tensor_tensor(qq[:], s[1][:], cprev[:],
                                        AluOpType.mult)
                pp = gatesp.tile([128, bl], CDT, tag=f"pp{li}",
                                 name=f"pp{li}", bufs=1)
                nc.vector.tensor_tensor(pp[:], s[0][:], s[2][:],
                                        AluOpType.mult)
                nc.vector.tensor_tensor(cnew[:], qq[:], pp[:], AluOpType.add)
                tc_ = gatesp.tile([128, bl], CDT, tag=f"tc{li}",
                                  name=f"tc{li}")
                nc.scalar.activation(tc_[:], cnew[:], AF.Tanh)
                nc.vector.tensor_tensor(hnew[:], s[3][:], tc_[:],
                                        AluOpType.mult)

            def sigma_set(ps, li, bias):
                s = {}
                for g in GATE_ORDER:
                    o = gatesp.tile([128, bl], CDT, tag=f"s{li}_{g}",
                                    name=f"s{li}_{g}")
                    fn = AF.Tanh if g == 2 else AF.Sigmoid
                    nc.scalar.activation(o[:], ps[g][:], fn, bias=bias(g))
                    s[g] = o
                return s

            def l1_compute(t):
                """sigma + tail for layer 1 at step t (one step behind
                layer 0 in emission order)."""
                s1 = sigma_set(g1_ps[0], 1, lambda g: b1c[:, g:g + 1])
                cell_tail(s1, c1s[t % 2][:], c1s[(t + 1) % 2][:],
                          h1[(t + 1) % 2][:], 1)

            def l1_stage(t):
                h1n = h1[(t + 1) % 2]
                pst = psumE.tile([128, nb, 128], CDT, tag="pst", name="pst")
                for q in range(nb):
                    nc.tensor.transpose(pst[:, q, :],
                                        h1n[:, q * 128:(q + 1) * 128],
                                        ident_b[:])
                nc.vector.tensor_copy(stE_ref[0][:, :, t % TC_E, :], pst[:])
                if t % TC_E == TC_E - 1:
                    t0 = t - (TC_E - 1)
                    nc.sync.dma_start(XEap[:, :, t0:t0 + TC_E, :],
                                      stE_ref[0][:])
                    if t + 1 < t_len:
                        stE_ref[0] = stageE.tile([128, nb, TC_E, 128], FP32,
                                                 tag="stE", name="stE")

            g0_cur = [None]
            g1_ps = [None]

            def lstm_all(bg_sched=()):
                # bg_sched: list of (step, thunk) sorted by step
                bg = list(bg_sched)
                g0_cur[0] = g0_ih(0)
                for t in range(t_len):
                    # finish layer-0 gates: hh matmuls into held psums
                    for g in GATE_ORDER:
                        nc.tensor.matmul(
                            g0_cur[0][g][:],
                            W["w0h"][:, g * 128:(g + 1) * 128],
                            h0[t % 2][:], start=False, stop=True,
                            skip_group_check=True)
                    s0 = sigma_set(g0_cur[0], 0,
                                   lambda g: biasT[:, g, t:t + 1])
                    cell_tail(s0, c0s[t % 2][:], c0s[(t + 1) % 2][:],
                              h0[(t + 1) % 2][:], 0)
                    # prefetch next step's ih matmuls
                    if t + 1 < t_len:
                        g0_cur[0] = g0_ih(t + 1)
                    # layer-1 sigma/tail for the PREVIOUS step: must be
                    # emitted before the g1(t) matmuls that read h1
                    if g1_ps[0] is not None:
                        l1_compute(t - 1)
                    # layer-1 matmuls for step t
                    nps = {}
                    for g in GATE_ORDER:
                        p = psump.tile([128, bl], FP32, tag="ps", name="ps1")
                        nc.tensor.matmul(p[:],
                                         W["w1i"][:, g * 128:(g + 1) * 128],
                                         h0[(t + 1) % 2][:], start=True,
                                         stop=False, skip_group_check=True)
                        nc.tensor.matmul(p[:],
                                         W["w1h"][:, g * 128:(g + 1) * 128],
                                         h1[t % 2][:], start=False, stop=True,
                                         skip_group_check=True)
                        nps[g] = p
                    if g1_ps[0] is not None:
                        l1_stage(t - 1)
                    g1_ps[0] = nps
                    while bg and bg[0][0] <= t:
                        bg.pop(0)[1]()
                l1_compute(t_len - 1)
                l1_stage(t_len - 1)

            t_q = t_len // 4
            for th in half_prologue(0, 0, t_q):
                th()
            n_xt = t_len // TC_X
            bg_all = []
            for u in range(1, 4):
                ths = half_prologue(u, u * t_q, (u + 1) * t_q,
                                    bulk_eng=nc.vector)
                # unit u must complete before step 32*u; spread its DVE work
                # across the whole preceding window so DVE never bursts
                start = max(3, 32 * (u - 1) + 6)
                span = max(1, (32 * u - 6 - start) // max(1, len(ths)))
                for j, th in enumerate(ths):
                    bg_all.append((start + j * span, th))
            for c0 in range(n_xt):
                bg_all.append((16 + c0 * (t_len - 22) // n_xt,
                               lambda c0=c0: xt_chunk(c0)))
            bg_all.sort(key=lambda x: x[0])
            lstm_all(bg_sched=bg_all)

    nc.compile()
    return nc


def host_prep(inputs, ncores=NCORES, bl=BL, t_len=T):
    X = np.ascontiguousarray(np.asarray(inputs["X"], dtype=np.float32))
    attn_w = np.asarray(inputs["attn_w"], dtype=np.float32)
    w_x = attn_w[2 * H:]
    wxb = np.ascontiguousarray(np.broadcast_to(w_x[None, :t_len], (128, t_len)))
    gamma_c = np.ascontiguousarray(
        np.asarray(inputs["bn_gamma"], np.float32).reshape(N, 1))
    beta_c = np.ascontiguousarray(
        np.asarray(inputs["bn_beta"], np.float32).reshape(N, 1))
    mats = {}
    for nm, key in (("w0i_t", "W_ih0"), ("w0h_t", "W_hh0"),
                    ("w1i_t", "W_ih1"), ("w1h_t", "W_hh1")):
        mats[nm] = np.ascontiguousarray(
            np.asarray(inputs[key], np.float32).T)
    b0 = (np.asarray(inputs["b_ih0"], np.float32)
          + np.asarray(inputs["b_hh0"], np.float32))
    b1 = (np.asarray(inputs["b_ih1"], np.float32)
          + np.asarray(inputs["b_hh1"], np.float32))
    b0_c = np.ascontiguousarray(b0.reshape(4, 128).T)
    b1_c = np.ascontiguousarray(b1.reshape(4, 128).T)

    in_maps = []
    for k in range(ncores):
        m = {
            "x_in": np.ascontiguousarray(X[k * bl:(k + 1) * bl, :t_len, :]),
            "wxb": wxb, "gamma_c": gamma_c, "beta_c": beta_c,
            "b0_c": b0_c, "b1_c": b1_c,
        }
        m.update(mats)
        in_maps.append(m)
    return in_maps


_NC_CACHE = {}


def _get_nc():
    if "nc" not in _NC_CACHE:
        _NC_CACHE["nc"] = build_nc()
    return _NC_CACHE["nc"]


def _get_fn():
    """Build (once) a cached sharded executable so repeated kernel() calls
    skip the per-call jit/compile of the run_bass_kernel_spmd path."""
    if "fn" in _NC_CACHE:
        return _NC_CACHE["fn"]
    import jax
    from jax.sharding import Mesh, PartitionSpec, NamedSharding
    from jax.experimental.shard_map import shard_map
    from concourse import bass2jax

    nc = _get_nc()
    bass2jax.install_neuronx_cc_hook()
    pname = nc.partition_id_tensor.name if nc.partition_id_tensor else None
    in_names, out_names, out_avals, zero_outs = [], [], [], []
    for alloc in nc.m.functions[0].allocations:
        if not isinstance(alloc, mybir.MemoryLocationSet):
            continue
        name = alloc.memorylocations[0].name
        if alloc.kind == "ExternalInput":
            if name != pname:
                in_names.append(name)
        elif alloc.kind == "ExternalOutput":
            shape = tuple(alloc.tensor_shape)
            dtype = mybir.dt.np(alloc.dtype)
            out_names.append(name)
            out_avals.append(jax.core.ShapedArray(shape, dtype))
            zero_outs.append(np.zeros(shape, dtype))
    all_in_names = list(in_names) + list(out_names)
    if pname is not None:
        all_in_names.append(pname)

    def _body(*args):
        operands = list(args)
        if pname is not None:
            operands.append(bass2jax.partition_id_tensor())
        outs = bass2jax._bass_exec_p.bind(
            *operands, out_avals=tuple(out_avals),
            in_names=tuple(all_in_names), out_names=tuple(out_names),
            lowering_input_output_aliases=(), sim_require_finite=True,
            sim_require_nnan=True, nc=nc)
        return tuple(outs)

    devices = jax.devices()[:NCORES]
    mesh = Mesh(np.asarray(devices), ("core",))
    nin = len(in_names) + len(out_names)
    fn = jax.jit(shard_map(_body, mesh=mesh,
                           in_specs=(PartitionSpec("core"),) * nin,
                           out_specs=(PartitionSpec("core"),) * len(out_names),
                           check_rep=False), keep_unused=True)
    sh = NamedSharding(mesh, PartitionSpec("core"))
    _NC_CACHE["fn"] = (fn, sh, in_names, out_names, zero_outs)
    return _NC_CACHE["fn"]


def kernel(**inputs):
    import jax
    fn, sh, in_names, out_names, zero_outs = _get_fn()
    in_maps = host_prep(inputs)
    args = []
    for nm in in_names:
        cat = np.concatenate([np.asarray(in_maps[c][nm])
                              for c in range(NCORES)], axis=0)
        args.append(jax.device_put(cat, sh))
    for z in zero_outs:
        cat = np.zeros((NCORES * z.shape[0], *z.shape[1:]), z.dtype)
        args.append(jax.device_put(cat, sh))
    outs = fn(*args)
    res = {nm: np.asarray(o) for nm, o in zip(out_names, outs)}
    return res["xt_out"], res["xe_out"]

